# revision 33
# baseline (speedup 1.0000x reference)
"""MultiHeadAttention TRN2 Bass kernel (all-bf16, host-side pre-transpose).

Full-input contract: kernel(**inputs) takes the unsharded tensors from
setup_inputs() and returns the full [4, 2048, 512] output.

Sharding: 8 cores = 4 batches x 2 query-halves. Each core computes its own
[1024, 512] slice of the output for one batch over all 8 heads, so the
gather is a pure concatenation (no collectives, no all-reduce).

Key hardware findings driving this design (vs the 408us v1 baseline):
  - The PE clock gate watches *row-group activity*: matmuls contracting
    only 64 partitions count ~half, and a stream of them never
    un-throttles the PE from its idle 1.2 GHz state. Score matmuls
    therefore contract the full 128 partitions, using per-head Q^T tiles
    whose other-head rows are zeroed (lhsT = whole two-head K^T tile).
  - The ACT (scalar) engine is the attention-phase floor: 16.8M exp
    elements at 1 elem/cycle/partition @1.2GHz ~= 128us. The AV matmuls
    lag the score/exp stream by one 2-chunk group so ACT stays ~100%
    busy while the PE idles ~15% (which also keeps the clock-gate
    activity average in the warm regime).
  - Host pre-transposes + pre-casts everything to bf16: x^T / W^T arrive
    DMA-ready, so there is no on-chip transpose phase at all.

Per-core pipeline:
  1. DMA W^T tiles (bf16) and x^T chunks (bf16) straight into SBUF.
  2. Projections: Q^T (per-head, zero-padded), K^T in [d_out, seq]
     layout; V in natural [seq, d_out] layout with a fused ones column
     (softmax denominators fall out of the AV matmul). Q/K bias-adds run
     on ACT (idle here), V epilogues on DVE. The V projection chunks
     interleave with head-0 score/exp groups to hide the attention
     pipeline fill.
  3. Per head, per 2-chunk group: scores -> ACT exp -> (one group later)
     AV accumulate; per-head reciprocal row-sums on DVE, split in
     halves, written into zero-padded broadcast staging tiles.
  4. Normalization via a single full-row selection matmul per head-pair
     + DVE multiply; out projection + bias -> DMA out.
"""
import contextlib

import numpy as np
import ml_dtypes

import bass_rust
import concourse.bass as bass
import concourse.mybir as mybir
import concourse.tile as tile
from concourse.bass_utils import run_bass_kernel_spmd
from concourse.tile import add_dep_helper

F32 = mybir.dt.float32
F32R = mybir.dt.float32r
BF16 = mybir.dt.bfloat16

B, S, D_MODEL = 4, 2048, 512
NUM_HEADS = 8
HEAD_DIM = 64
SQ = S // 2  # queries per core
N_CORES = 8
SCALE = 1.0 / 8.0  # 1/sqrt(HEAD_DIM)

_split_ctr = [0]


def split_waits(nc, max_waits: int = 1):
    """walrus codegen rejects instructions carrying >1 sync wait; move the
    extras onto standalone EventSemaphore instructions on the same engine."""
    for f in nc.m.functions:
        for blk in f.blocks:
            new_insts = []
            changed = False
            for inst in blk.instructions:
                si = inst.sync_info
                if si is not None and si.on_wait and len(si.on_wait) > max_waits:
                    waits = list(si.on_wait)
                    extra, keep = waits[:-max_waits], waits[-max_waits:]
                    for w in extra:
                        _split_ctr[0] += 1
                        ev = mybir.InstEventSemaphore(
                            name=f"I-wsplit-{_split_ctr[0]}", ins=[], outs=[]
                        )
                        ev.engine = inst.engine
                        ev.sync_info = bass_rust.SyncInfo(on_wait=[w], on_update=[])
                        new_insts.append(ev)
                    inst.sync_info = bass_rust.SyncInfo(
                        on_wait=keep, on_update=list(si.on_update)
                    )
                    changed = True
                new_insts.append(inst)
            if changed:
                blk.instructions = new_insts


def build_mha():
    nc = bass.Bass("TRN2", target_bir_lowering=False, debug=False, num_devices=1)

    # x^T inputs: [d_model, seq] bf16, host-transposed
    qd = nc.declare_dram_parameter("qt", [D_MODEL, SQ], BF16, isOutput=False).ap()
    kd = nc.declare_dram_parameter("kt", [D_MODEL, S], BF16, isOutput=False).ap()
    vd = nc.declare_dram_parameter("vt", [D_MODEL, S], BF16, isOutput=False).ap()
    # W^T weights: [d_in, d_out] bf16, host-transposed
    wts = {
        n: nc.declare_dram_parameter(n, [D_MODEL, D_MODEL], BF16, isOutput=False).ap()
        for n in ("wq", "wk", "wv", "wo")
    }
    bias = {
        n: nc.declare_dram_parameter(n, [D_MODEL], F32, isOutput=False).ap()
        for n in ("bq", "bk", "bv", "bo")
    }
    outd = nc.declare_dram_parameter("out", [SQ, D_MODEL], F32, isOutput=True).ap()

    H2 = NUM_HEADS // 2  # head pairs = dout tiles of 128
    KTILES = S // 128  # 16
    NGROUPS = KTILES // 2  # 2-chunk score/exp/AV groups per head

    with tile.TileContext(nc) as tc, contextlib.ExitStack() as top:
        consts = top.enter_context(tc.tile_pool(name="consts", bufs=1))
        wt_pool = top.enter_context(tc.tile_pool(name="wt", bufs=1))
        proj_out = top.enter_context(tc.tile_pool(name="proj_out", bufs=1))
        epilog = top.enter_context(tc.tile_pool(name="epilog", bufs=1))
        # scores psum: 2 tiles x 2 banks, used for the whole kernel
        ps_s = top.enter_context(tc.tile_pool(name="ps_s", bufs=2, space="PSUM"))

        # ---- constants
        # per-partition bias tiles for Q/K (bias indexed by d_out partition)
        bqt = consts.tile([128, 4], F32)
        bkt = consts.tile([128, 4], F32)
        for t_, name in ((bqt, "bq"), (bkt, "bk")):
            nc.gpsimd.dma_start(
                out=t_, in_=bias[name].rearrange("(c p) -> p c", p=128)
            )
        # free-dim broadcast biases for V / out
        bvb = consts.tile([128, D_MODEL], F32)
        bob = consts.tile([128, D_MODEL], F32)
        for t_, name in ((bvb, "bv"), (bob, "bo")):
            src = bias[name]
            nc.gpsimd.dma_start(
                out=t_,
                in_=bass.AP(tensor=src.tensor, offset=src.offset, ap=[[0, 128], [1, D_MODEL]]),
            )
        ones8 = consts.tile([128, NUM_HEADS], BF16)
        nc.vector.memset(ones8, 1.0)
        # selection matrix for the rowsum broadcast: row 0 -> out rows 0-63
        # (even head of the pair), row 1 -> out rows 64-127 (odd head).
        # Full 128-partition contraction keeps the PE clock gate warm.
        # (row 32, not 1: engine APs need 32-aligned partition bases)
        sel_np = np.zeros((128, 128), np.float32)
        sel_np[0, 0:HEAD_DIM] = 1.0
        sel_np[32, HEAD_DIM:128] = 1.0
        sel_dram = nc.inline_tensor(sel_np, name="sel_const")
        sel_f = consts.tile([128, 128], F32)
        nc.gpsimd.dma_start(out=sel_f, in_=sel_dram.ap())
        sel = consts.tile([128, 128], F32R)
        nc.vector.tensor_copy(sel, sel_f)

        # ---- W^T tiles: straight DMA (host transposed + cast).
        # WT[n][:, dc, :] = W^T[dc*128:(dc+1)*128, :]  (partition = d_in)
        WT = {
            name: wt_pool.tile([128, 4, D_MODEL], BF16, name=f"wt_{name}", tag=f"wt_{name}")
            for name in wts
        }

        def load_wt(name):
            # split per [128, 256] piece so more DMA queues run in parallel
            # (shortens the critical path to the first projection matmul)
            wsrc = wts[name].rearrange("(c p) m -> p c m", p=128)
            for dc in range(4):
                for mh in range(2):
                    msl = slice(mh * 256, (mh + 1) * 256)
                    nc.sync.dma_start(
                        out=WT[name][:, dc, msl], in_=wsrc[:, dc, msl]
                    )

        # ---- long-lived activation tiles
        # QTZ[h]: per-head Q^T with the OTHER head's 64 rows zeroed, so the
        # score matmul contracts the full 128 partitions (lhsT = whole
        # two-head KT tile; zero rows null the other head's contribution).
        QTZ = [
            proj_out.tile([128, SQ], BF16, name=f"qtz_{h}", tag=f"qtz_{h}")
            for h in range(NUM_HEADS)
        ]
        for h in range(NUM_HEADS):
            half = h % 2
            nc.vector.memset(QTZ[h][(1 - half) * HEAD_DIM : (2 - half) * HEAD_DIM, :], 0.0)
        KT = [proj_out.tile([128, S], BF16, name=f"kt_{t}", tag=f"kt_{t}") for t in range(H2)]
        V = [
            proj_out.tile([128, NUM_HEADS, HEAD_DIM + 1], BF16, name=f"v_{sc}", tag=f"v_{sc}")
            for sc in range(KTILES)
        ]
        OU = [epilog.tile([128, SQ], BF16, name=f"ou_{t}", tag=f"ou_{t}") for t in range(H2)]
        OMT = [
            epilog.tile([128, SQ], BF16, name=f"omt_{t}", tag=f"omt_{t}")
            for t in range(H2)
        ]
        # RSZ[t]: broadcast staging for reciprocal row-sums; rows 0/32 carry
        # heads 2t/2t+1, other rows are zeroed (multiplied by sel's zero
        # rows, but must not hold NaN garbage).
        RSZ = [
            epilog.tile([128, SQ], F32R, name=f"rsz_{t}", tag=f"rsz_{t}")
            for t in range(H2)
        ]
        for t in range(H2):
            # memset rejects f32r; the f32 bitcast view is identical bits
            nc.vector.memset(RSZ[t].bitcast(F32), 0.0)

        pe_chain = [None]

        def chain(bi):
            if pe_chain[0] is not None:
                add_dep_helper(bi.ins, pe_chain[0].ins, reason="pe-order")
            pe_chain[0] = bi

        # ---- attention-group emitters ------------------------------------
        eh_of = {}
        po_of = {}
        ps_o_pool = [None]
        ehpool_ref = [None]

        def emit_scores(h, kb):
            """Score matmuls + exp for group (h, kb); returns nothing.
            eh tile for head h is created on first use."""
            if h not in eh_of:
                eh_of[h] = ehpool_ref[0].tile(
                    [128, KTILES, SQ], BF16, name=f"eh_{h}", tag="eh"
                )
            eh = eh_of[h]
            pss = []
            for j in range(2):
                kc = 2 * kb + j
                pscore = ps_s.tile([128, SQ], F32, tag="pscore")
                for qc in range(SQ // 512):
                    sl = slice(qc * 512, (qc + 1) * 512)
                    chain(
                        nc.tensor.matmul(
                            pscore[:, sl],
                            KT[h // 2][:, kc * 128 : (kc + 1) * 128],
                            QTZ[h][:, sl],
                            start=True,
                            stop=True,
                        )
                    )
                pss.append((kc, pscore))
            for kc, pscore in pss:
                nc.scalar.activation(
                    eh[:, kc, :],
                    pscore,
                    mybir.ActivationFunctionType.Exp,
                    scale=SCALE,
                )

        def emit_av(h, kb):
            """AV matmuls for group (h, kb): consume that group's exp
            output. Runs behind the score/exp stream so ACT stays
            saturated (the attention-phase bottleneck at ~2.1us/group)."""
            if h not in po_of:
                po_of[h] = ps_o_pool[0].tile(
                    [HEAD_DIM + 1, SQ], F32, name=f"po_{h}", tag="po"
                )
            po, eh = po_of[h], eh_of[h]
            for j in range(2):
                kc = 2 * kb + j
                for qc in range(SQ // 512):
                    sl = slice(qc * 512, (qc + 1) * 512)
                    chain(
                        nc.tensor.matmul(
                            po[:, sl],
                            V[kc][:, h, :],
                            eh[:, kc, sl],
                            start=(kc == 0),
                            stop=(kc == KTILES - 1),
                        )
                    )
            if kb == NGROUPS - 1:
                finalize(h)

        def finalize(h):
            t, half = h // 2, h % 2
            po = po_of.pop(h)
            # per-query-half pipeline: reciprocal first (it gates the
            # broadcast matmul + normalize chain in the tail), then the
            # numerator copy. f32r output is bit-identical to f32.
            for qh in range(2):
                sl = slice(qh * 512, (qh + 1) * 512)
                with nc.allow_low_precision(reason="f32r is fp32-bit-compatible"):
                    nc.vector.reciprocal(
                        RSZ[t][32 * half : 32 * half + 1, sl],
                        po[HEAD_DIM : HEAD_DIM + 1, sl],
                    )
                nc.vector.tensor_copy(
                    OU[t][half * HEAD_DIM : (half + 1) * HEAD_DIM, sl],
                    po[0:HEAD_DIM, sl],
                )

        # ================= phase 1: projections ===========================
        with tc.tile_pool(name="ehpool", bufs=2) as ehpool:
            ehpool_ref[0] = ehpool
            with (
                tc.tile_pool(name="xt", bufs=3) as xt_pool,
                tc.tile_pool(name="pp", bufs=3, space="PSUM") as pp,
            ):
                def load_xt(src_ap, s0):
                    """DMA a [512, 512] chunk of x^T (cols s0:s0+512) into a
                    [128, 4, 512] tile; 4 DMAs so queues parallelize."""
                    xt_c = xt_pool.tile([128, 4, 512], BF16, tag="xt")
                    xsrc = src_ap.rearrange("(c p) s -> p c s", p=128)
                    for dc in range(4):
                        nc.sync.dma_start(
                            out=xt_c[:, dc, :], in_=xsrc[:, dc, s0 : s0 + 512]
                        )
                    return xt_c

                # Q^T and K^T: transposed-layout projections.
                # Bias epilogues run on ACT (idle during projections).
                for src_ap, wname, bt, slen in (
                    (qd, "wq", bqt, SQ),
                    (kd, "wk", bkt, S),
                ):
                    load_wt(wname)
                    for c in range(slen // 512):
                        xt_c = load_xt(src_ap, c * 512)
                        for t in range(H2):
                            pj = pp.tile([128, 512], F32, tag="pproj")
                            for dc in range(4):
                                chain(
                                    nc.tensor.matmul(
                                        pj,
                                        WT[wname][:, dc, t * 128 : (t + 1) * 128],
                                        xt_c[:, dc, :],
                                        start=(dc == 0),
                                        stop=(dc == 3),
                                    )
                                )
                            sl = slice(c * 512, (c + 1) * 512)
                            if wname == "wq":
                                # split head-pair into per-head zero-padded
                                # tiles (partition-aligned halves)
                                for half in range(2):
                                    hsl = slice(half * HEAD_DIM, (half + 1) * HEAD_DIM)
                                    nc.scalar.add(
                                        QTZ[2 * t + half][hsl, sl],
                                        pj[hsl, :],
                                        bt[hsl, t : t + 1],
                                    )
                            else:
                                nc.scalar.add(KT[t][:, sl], pj, bt[:, t : t + 1])

                # V projection (natural layout, fused ones column),
                # interleaved with ALL of head-0's score/exp groups (two per
                # V chunk): the score matmuls wait on exp-drained psum
                # slots, and the V matmuls fill those PE bubbles while ACT
                # ramps up, hiding the whole first head under the
                # projection phase.
                load_wt("wv")
                load_wt("wo")  # needed only in phase 4; issue last
                for c in range(S // 512):
                    emit_scores(0, 2 * c)
                    xt_c = load_xt(vd, c * 512)
                    for st in range(4):
                        sc = c * 4 + st
                        pj = pp.tile([128, 512], F32, tag="pproj")
                        for dc in range(4):
                            chain(
                                nc.tensor.matmul(
                                    pj,
                                    xt_c[:, dc, st * 128 : (st + 1) * 128],
                                    WT["wv"][:, dc, :],
                                    start=(dc == 0),
                                    stop=(dc == 3),
                                )
                            )
                        pj3 = pj.rearrange("p (h d) -> p h d", h=NUM_HEADS)
                        nc.vector.tensor_add(
                            V[sc][:, :, 0:HEAD_DIM],
                            pj3,
                            bvb.rearrange("p (h d) -> p h d", h=NUM_HEADS),
                        )
                        nc.vector.tensor_copy(
                            V[sc][:, :, HEAD_DIM : HEAD_DIM + 1],
                            ones8.rearrange("p (h o) -> p h o", o=1),
                        )
                    emit_scores(0, 2 * c + 1)

            # ================= phase 3: attention =========================
            with tc.tile_pool(name="ps_o", bufs=2, space="PSUM") as ps_o:
                ps_o_pool[0] = ps_o

                def bcast_mul(t, pr_pool, pr_tag, qhs=(0, 1), fuse_mul=True):
                    """Broadcast RSZ[t] -> [128, SQ] reciprocal field and
                    multiply into OMT[t]."""
                    pr = pr_pool.tile([128, SQ], F32, name=f"pr_{t}", tag=pr_tag)
                    for qc in qhs:
                        sl = slice(qc * 512, (qc + 1) * 512)
                        chain(
                            nc.tensor.matmul(
                                pr[:, sl], sel, RSZ[t][:, sl], start=True, stop=True
                            )
                        )
                        if fuse_mul:
                            nc.vector.tensor_mul(OMT[t][:, sl], OU[t][:, sl], pr[:, sl])

                # head-0 groups all pre-emitted above (no AV yet)
                pending = [(0, kb) for kb in range(NGROUPS)]
                for h in range(1, NUM_HEADS):
                    for kb in range(NGROUPS):
                        emit_scores(h, kb)
                        pending.append((h, kb))
                        emit_av(*pending.pop(0))
                        # drain the initial backlog two-at-a-time until the
                        # steady-state lag of one group is reached
                        if len(pending) > 1:
                            emit_av(*pending.pop(0))
                # pairs 0-2 are finalized well before the stream ends:
                # normalize them here so the tail only handles pair 3
                # (emitted before the last AVs so the DVE multiplies queue
                # ahead of head 7's reciprocals)
                bcast_mul(0, ps_s, "pscore")
                bcast_mul(1, ps_s, "pscore")
                bcast_mul(2, ps_s, "pscore")
                while pending:
                    emit_av(*pending.pop(0))

        # ================= phase 4: pair-3 normalize + out projection =====
        # Pair 3 (heads 6/7) finishes last: pipeline it per query-half so
        # the out-projection for the first half starts while head 7's
        # second-half reciprocal is still running.
        with (
            tc.tile_pool(name="outsb", bufs=3) as outsb,
            tc.tile_pool(name="ps_f", bufs=2, space="PSUM") as ps_f,
        ):
            for qh in range(2):
                sl = slice(qh * 512, (qh + 1) * 512)
                pr3 = ps_s.tile([128, 512], F32, name=f"pr3_{qh}", tag="pscore")
                chain(
                    nc.tensor.matmul(pr3, sel, RSZ[3][:, sl], start=True, stop=True)
                )
                nc.vector.tensor_mul(OMT[3][:, sl], OU[3][:, sl], pr3)
                for sq in range(qh * 4, qh * 4 + 4):
                    pf = ps_f.tile([128, D_MODEL], F32, tag="pf")
                    for t in range(H2):
                        chain(
                            nc.tensor.matmul(
                                pf,
                                OMT[t][:, sq * 128 : (sq + 1) * 128],
                                WT["wo"][:, t, :],
                                start=(t == 0),
                                stop=(t == H2 - 1),
                            )
                        )
                    ot = outsb.tile([128, D_MODEL], F32, tag="ot")
                    nc.vector.tensor_add(ot, pf, bob)
                    nc.sync.dma_start(out=outd[sq * 128 : (sq + 1) * 128, :], in_=ot)

    split_waits(nc)
    return nc


_cached_nc = None


def _get_nc():
    global _cached_nc
    if _cached_nc is None:
        _cached_nc = build_mha()
    return _cached_nc


def build_in_maps(q, k, v, Wq, bq, Wk, bk, Wv, bv, Wo, bo):
    """Host-side prep: cast to bf16 and pre-transpose x and W so the device
    sees x^T / W^T directly (shared by kernel() and test harness)."""
    bf16 = ml_dtypes.bfloat16
    q = np.asarray(q, dtype=np.float32)
    k = np.asarray(k, dtype=np.float32)
    v = np.asarray(v, dtype=np.float32)
    weights = {
        "wq": np.ascontiguousarray(np.asarray(Wq, np.float32).T.astype(bf16)),
        "wk": np.ascontiguousarray(np.asarray(Wk, np.float32).T.astype(bf16)),
        "wv": np.ascontiguousarray(np.asarray(Wv, np.float32).T.astype(bf16)),
        "wo": np.ascontiguousarray(np.asarray(Wo, np.float32).T.astype(bf16)),
        "bq": np.ascontiguousarray(np.asarray(bq, np.float32)),
        "bk": np.ascontiguousarray(np.asarray(bk, np.float32)),
        "bv": np.ascontiguousarray(np.asarray(bv, np.float32)),
        "bo": np.ascontiguousarray(np.asarray(bo, np.float32)),
    }
    in_maps = []
    for core in range(N_CORES):
        b, qh = core // 2, core % 2
        in_maps.append(
            {
                "qt": np.ascontiguousarray(
                    q[b, qh * SQ : (qh + 1) * SQ, :].T.astype(bf16)
                ),
                "kt": np.ascontiguousarray(k[b].T.astype(bf16)),
                "vt": np.ascontiguousarray(v[b].T.astype(bf16)),
                **weights,
            }
        )
    return in_maps


def kernel(q, k, v, mask, Wq, bq, Wk, bk, Wv, bv, Wo, bo, **_unused):
    in_maps = build_in_maps(q, k, v, Wq, bq, Wk, bk, Wv, bv, Wo, bo)
    nc = _get_nc()
    res = run_bass_kernel_spmd(nc, in_maps, list(range(N_CORES)))
    out = np.empty((B, S, D_MODEL), dtype=np.float32)
    for core in range(N_CORES):
        b, qh = core // 2, core % 2
        out[b, qh * SQ : (qh + 1) * SQ, :] = res.results[core]["out"]
    return out
